# revision 12
# baseline (speedup 1.0000x reference)
"""Elman RNN (return_sequences=False) on 8 TRN2 NeuronCores (bass, raw bacc).

Per core (batch shard of 32, all on-chip tensors transposed [feature, batch]):
  - x host-permuted to [D, T*Bs] d-major -> full-bandwidth contiguous DMA
  - proj^T = w^T @ x for 16 steps at a time lands in one PSUM bank (N=512)
  - each step: PE accumulates sw^T @ s into its 32-col PSUM slice
    (start=False), ACT computes tanh(psum + bias) -> next state tile
  - matmuls in bf16 (fp32 lowers to 2 half-speed passes); tanh/PSUM fp32

Raw bacc with manual semaphores so every critical instruction carries its
single cross-engine wait itself (no standalone per-step EVENT_SEMAPHORE),
and the recurrence matmuls skip their weight reload (ldweights=False; the
stationary weights are restored once per 16-step bank, off the chain).
Steady state measured on silicon: 560 ns/step =
MATMUL 184 + sem 37 + ACTIVATE 287 + sem 52.

Sync scheme (per core):
  s_x0/s_x1 : +16 per x-chunk DMA (per buffer parity; HWDGE transfers on one
              queue can complete out of order, so parity sems disambiguate)
  s_dma     : +16 per const DMA (w, sw, b)
  s_proj    : +1 per proj matmul (64 total, 1 per 16-step bank)
  s_pe      : +1 per recurrence matmul (1023 total)
  s_act     : +1 per ACTIVATE (1024 total)

Waits:
  PE head:   s_dma >= 48             [w/sw resident before any weight load]
  proj MM b: s_x[par] >= 16*(c//2+1) [x chunk c=b//4 loaded]
             (bank-reuse WAR vs s_act is structurally slack: proj(b) sits in
              the PE stream two blocks early, where ACT count >> required)
  rec MM t:  s_act >= t              [state t-1 written]
  ACT t:     s_pe >= t (t>0) / s_proj >= 1 (t=0)
  DMA x_c (c>=2): s_proj >= 4*(c-1)  [x slot c-2 consumed]
  DMA out:   s_act >= 1024
"""

from contextlib import ExitStack

import numpy as np
import ml_dtypes

import concourse.bass as bass
import concourse.bacc as bacc
from concourse import mybir

B, T, D, H = 256, 1024, 128, 128
NCORES = 8
BS = B // NCORES
F32 = mybir.dt.float32
BF16 = mybir.dt.bfloat16

BLK_T = 16      # steps per PSUM bank
CHUNK_T = 64    # steps per x DMA chunk (4 banks)
NSTATE = 4      # rotating state buffers


def build(T_=T):
    nblk = T_ // BLK_T
    nchunk = T_ // CHUNK_T
    tanh = mybir.ActivationFunctionType.Tanh

    nc = bacc.Bacc("TRN2", target_bir_lowering=False, debug=False,
                   num_devices=NCORES)
    x_d = nc.dram_tensor("x", [D, T_ * BS], BF16, kind="ExternalInput")
    w_d = nc.dram_tensor("w", [D, H], BF16, kind="ExternalInput")
    sw_d = nc.dram_tensor("sw", [H, H], BF16, kind="ExternalInput")
    b_d = nc.dram_tensor("b", [H, 1], F32, kind="ExternalInput")
    out_d = nc.dram_tensor("out", [H, BS], F32, kind="ExternalOutput")

    ctx = ExitStack()
    with ctx:
        w_sb = ctx.enter_context(nc.sbuf_tensor("w_sb", [D, H], BF16))
        sw_sb = ctx.enter_context(nc.sbuf_tensor("sw_sb", [H, H], BF16))
        b_sb = ctx.enter_context(nc.sbuf_tensor("b_sb", [H, 1], F32))
        xbuf = [ctx.enter_context(
            nc.sbuf_tensor(f"xbuf{i}", [D, CHUNK_T * BS], BF16))
            for i in range(2)]
        st = [ctx.enter_context(nc.sbuf_tensor(f"st{i}", [H, BS], BF16))
              for i in range(NSTATE)]
        st_f = ctx.enter_context(nc.sbuf_tensor("st_f", [H, BS], F32))
        psum = ctx.enter_context(nc.psum_tensor("psum", [H, 4096], F32))

        s_dma = ctx.enter_context(nc.semaphore("s_dma"))
        s_x0 = ctx.enter_context(nc.semaphore("s_x0"))
        s_x1 = ctx.enter_context(nc.semaphore("s_x1"))
        s_proj = ctx.enter_context(nc.semaphore("s_proj"))
        s_pe = ctx.enter_context(nc.semaphore("s_pe"))
        s_act = ctx.enter_context(nc.semaphore("s_act"))
        s_x = [s_x0, s_x1]

        def pslice(t):
            blk = t // BLK_T
            return psum[:, (blk % 8) * 512 + (t % BLK_T) * BS:
                        (blk % 8) * 512 + (t % BLK_T) * BS + BS]

        with nc.Block() as block:
            @block.sync
            def _(sync):
                # x0 first: it's the largest head transfer and gates the
                # first proj; consts stream concurrently behind it. The PE
                # stream waits for s_dma>=48 separately before touching w/sw.
                for c in range(nchunk):
                    if c >= 2:
                        # x slot c-2 fully consumed; also orders same-parity
                        # transfers (chunk c-1's projs waited on x_{c-1},
                        # which waited on... chunk c-2 complete)
                        sync.wait_ge(s_proj, 4 * (c - 1))
                    sync.dma_start(
                        xbuf[c % 2][:],
                        x_d.ap()[:, c * CHUNK_T * BS:(c + 1) * CHUNK_T * BS],
                    ).then_inc(s_x[c % 2], 16)
                    if c == 0:
                        sync.dma_start(w_sb[:], w_d.ap()).then_inc(s_dma, 16)
                        sync.dma_start(sw_sb[:], sw_d.ap()).then_inc(s_dma, 16)
                        sync.dma_start(b_sb[:], b_d.ap()).then_inc(s_dma, 16)
                sync.wait_ge(s_act, T_)
                sync.dma_start(out_d.ap(), st_f[:]).then_inc(s_dma, 16)

            @block.tensor
            def _(tensor):
                def proj(b):
                    c = b // 4
                    tensor.wait_ge(s_x[c % 2], 16 * (c // 2 + 1))
                    xb = xbuf[c % 2]
                    off = (b % 4) * BLK_T * BS
                    bank = (b % 8) * 512
                    tensor.matmul(psum[:, bank:bank + 512], w_sb[:],
                                  xb[:, off:off + BLK_T * BS],
                                  start=True, stop=False,
                                  skip_group_check=True,
                                  ).then_inc(s_proj, 1)

                tensor.wait_ge(s_dma, 48)  # w/sw resident before any LDW
                proj(0)
                proj(1)
                for t in range(T_):
                    if t % BLK_T == 0:
                        if t // BLK_T + 2 < nblk:
                            proj(t // BLK_T + 2)
                        # restore the recurrence weights after proj clobbered
                        # the array; runs in the ACT window, off the chain
                        tensor.ldweights(sw_sb[:])
                    if t > 0:
                        tensor.wait_ge(s_act, t)
                        mm = tensor.matmul(pslice(t), sw_sb[:],
                                           st[(t - 1) % NSTATE][:],
                                           start=False,
                                           stop=(t % BLK_T == BLK_T - 1),
                                           skip_group_check=True)
                        # weights already resident: suppress the per-step
                        # LDWEIGHTS so the sem wait rides on the MATMUL
                        mm.ins.ldweights = False
                        mm.then_inc(s_pe, 1)

            @block.scalar
            def _(scalar):
                for t in range(T_):
                    if t == 0:
                        scalar.wait_ge(s_proj, 1)
                    else:
                        scalar.wait_ge(s_pe, t)
                    dst = st_f if t == T_ - 1 else st[t % NSTATE]
                    scalar.activation(dst[:], pslice(t), tanh,
                                      bias=b_sb[:, 0:1]).then_inc(s_act, 1)

    # Keep per-step waits on the MATMUL, not the generated LDWEIGHTS: the
    # recurrence weights are loop constants, so the weight load is safe to
    # run early (during the previous tanh) instead of after the sem clears.
    nc.move_matmul_waits_to_ldweights = lambda: None
    nc.compile()
    return nc


def shard_inputs(x, w, state_weight, b):
    x = np.asarray(x)
    w = np.ascontiguousarray(np.asarray(w).astype(ml_dtypes.bfloat16))
    sw = np.ascontiguousarray(np.asarray(state_weight).astype(ml_dtypes.bfloat16))
    bb = np.ascontiguousarray(np.asarray(b), dtype=np.float32).reshape(H, 1)
    in_maps = []
    for i in range(NCORES):
        xs = np.asarray(x[i * BS:(i + 1) * BS])
        xs = np.ascontiguousarray(xs.transpose(2, 1, 0).astype(ml_dtypes.bfloat16))
        in_maps.append({"x": xs.reshape(D, -1), "w": w, "sw": sw, "b": bb})
    return in_maps


_NC = None


def kernel(x, w, state_weight, b, **run_kwargs):
    global _NC
    from concourse.bass_utils import run_bass_kernel_spmd
    if _NC is None:
        _NC = build()
    in_maps = shard_inputs(x, w, state_weight, b)
    res = run_bass_kernel_spmd(_NC, in_maps, core_ids=list(range(NCORES)),
                               **run_kwargs)
    out = np.concatenate([r["out"].T for r in res.results], axis=0)
    if run_kwargs:
        return out, res
    return out


# revision 13
# speedup vs baseline: 1.0006x; 1.0006x over previous
"""Elman RNN (return_sequences=False) on 8 TRN2 NeuronCores (raw bass/bacc).

Reference math:  proj = x @ w + b;  s[0] = tanh(proj[0]);
                 s[t] = tanh(proj[t] + s[t-1] @ state_weight);  out = s[T-1].

Sharding: data-parallel over batch (32 rows/core), weights replicated, no
collectives; host gathers by concatenation. All on-chip tensors live
transposed ([feature, batch]) so the contraction dim is always the SBUF
partition dim and no device-side transposes are needed; x is host-permuted
per core to d-major [D, T*Bs] for full-bandwidth contiguous DMA.

Structure per core:
  - proj^T for 16 steps at a time is accumulated straight into one PSUM
    bank as x_hi@w_hi + x_hi@w_lo + x_lo@w_hi (split-bf16: v_hi = bf16(v),
    v_lo = bf16(v - v_hi)), giving ~f32-class GEMM error at bf16 speed.
    The 6 N=256 sub-matmuls per bank hide in the recurrence's PE idle
    windows, two blocks ahead of use.
  - each step: PE accumulates sw^T @ s into its 32-col PSUM slice
    (start=False), ACT computes tanh(psum + bias) into the next bf16 state
    tile. The serial chain is latency-bound; measured steady state is
    560 ns/step = MATMUL 184 + sem 37 + ACTIVATE 287 + sem 52.
  - raw semaphores so each critical instruction carries its single
    cross-engine wait itself (no per-step standalone EVENT_SEMAPHORE), and
    the recurrence matmuls skip their weight reload (ldweights=False; the
    stationary weights are restored once per bank, off the chain).

End-to-end on silicon: ~592 us, max rel err ~3.3e-3 (bf16 state
quantization floor; fp32 everywhere measures 1177 us at 4.6e-7).
"""

from contextlib import ExitStack

import numpy as np
import ml_dtypes

import concourse.bass as bass
import concourse.bacc as bacc
from concourse import mybir

B, T, D, H = 256, 1024, 128, 128
NCORES = 8
BS = B // NCORES
F32 = mybir.dt.float32
BF16 = mybir.dt.bfloat16

BLK_T = 16      # steps per PSUM bank
CHUNK_T = 64    # steps per x DMA chunk (4 banks)
NSTATE = 4      # rotating state buffers


def build(T_=T):
    nblk = T_ // BLK_T
    nchunk = T_ // CHUNK_T
    tanh = mybir.ActivationFunctionType.Tanh

    nc = bacc.Bacc("TRN2", target_bir_lowering=False, debug=False,
                   num_devices=NCORES)
    # x packed as [D, 2, T*Bs]: plane 0 = x_hi, plane 1 = x_lo
    x_d = nc.dram_tensor("x", [D, 2, T_ * BS], BF16, kind="ExternalInput")
    # w packed as [D, 2, H]: plane 0 = w_hi, plane 1 = w_lo
    w_d = nc.dram_tensor("w", [D, 2 * H], BF16, kind="ExternalInput")
    sw_d = nc.dram_tensor("sw", [H, H], BF16, kind="ExternalInput")
    b_d = nc.dram_tensor("b", [H, 1], F32, kind="ExternalInput")
    out_d = nc.dram_tensor("out", [H, BS], F32, kind="ExternalOutput")

    ctx = ExitStack()
    with ctx:
        w_sb = ctx.enter_context(nc.sbuf_tensor("w_sb", [D, 2 * H], BF16))
        sw_sb = ctx.enter_context(nc.sbuf_tensor("sw_sb", [H, H], BF16))
        b_sb = ctx.enter_context(nc.sbuf_tensor("b_sb", [H, 1], F32))
        xbuf = [ctx.enter_context(
            nc.sbuf_tensor(f"xbuf{i}", [D, 2 * CHUNK_T * BS], BF16))
            for i in range(2)]
        st = [ctx.enter_context(nc.sbuf_tensor(f"st{i}", [H, BS], BF16))
              for i in range(NSTATE)]
        st_f = ctx.enter_context(nc.sbuf_tensor("st_f", [H, BS], F32))
        psum = ctx.enter_context(nc.psum_tensor("psum", [H, 4096], F32))

        s_dma = ctx.enter_context(nc.semaphore("s_dma"))
        s_x0 = ctx.enter_context(nc.semaphore("s_x0"))
        s_x1 = ctx.enter_context(nc.semaphore("s_x1"))
        s_proj = ctx.enter_context(nc.semaphore("s_proj"))
        s_pe = ctx.enter_context(nc.semaphore("s_pe"))
        s_act = ctx.enter_context(nc.semaphore("s_act"))
        s_x = [s_x0, s_x1]

        def pslice(t):
            blk = t // BLK_T
            return psum[:, (blk % 8) * 512 + (t % BLK_T) * BS:
                        (blk % 8) * 512 + (t % BLK_T) * BS + BS]

        with nc.Block() as block:
            @block.sync
            def _(sync):
                for c in range(nchunk):
                    if c >= 2:
                        sync.wait_ge(s_proj, 24 * (c - 1))
                    sync.dma_start(
                        xbuf[c % 2][:].rearrange("d (two n) -> d two n",
                                                 two=2),
                        x_d.ap()[:, :,
                                 c * CHUNK_T * BS:(c + 1) * CHUNK_T * BS],
                    ).then_inc(s_x[c % 2], 16)
                    if c == 0:
                        sync.dma_start(w_sb[:], w_d.ap()).then_inc(s_dma, 16)
                        sync.dma_start(sw_sb[:], sw_d.ap()).then_inc(s_dma, 16)
                        sync.dma_start(b_sb[:], b_d.ap()).then_inc(s_dma, 16)
                sync.wait_ge(s_act, T_)
                sync.dma_start(out_d.ap(), st_f[:]).then_inc(s_dma, 16)

            @block.tensor
            def _(tensor):
                HALF = BLK_T * BS // 2  # 256 cols

                def proj_piece(b, piece):
                    # piece 0..5: (term, half) = (piece//2, piece%2)
                    # terms: 0 = w_hi@x_hi, 1 = w_lo@x_hi, 2 = w_hi@x_lo
                    term, half = piece // 2, piece % 2
                    c = b // 4
                    tensor.wait_ge(s_x[c % 2], 16 * (c // 2 + 1))
                    xb = xbuf[c % 2]
                    xplane = CHUNK_T * BS if term == 2 else 0
                    wplane = H if term == 1 else 0
                    off = xplane + (b % 4) * BLK_T * BS + half * HALF
                    bank = (b % 8) * 512 + half * HALF
                    # only the bank's first touch carries start=True: it
                    # marks the whole 2KB zero region pending, so the other
                    # half's first write (piece 1) lands as a fresh value
                    # and later terms accumulate
                    tensor.matmul(psum[:, bank:bank + HALF],
                                  w_sb[:, wplane:wplane + H],
                                  xb[:, off:off + HALF],
                                  start=(piece == 0), stop=False,
                                  skip_group_check=True,
                                  ).then_inc(s_proj, 1)

                tensor.wait_ge(s_dma, 48)
                for b in range(2):
                    for p in range(6):
                        proj_piece(b, p)
                for t in range(T_):
                    k = t % BLK_T
                    bnext = t // BLK_T + 2
                    if k == 0 and bnext < nblk:
                        # hi@hi for both halves first (they must carry
                        # start=True before the accumulating terms)
                        proj_piece(bnext, 0)
                        proj_piece(bnext, 1)
                        tensor.ldweights(sw_sb[:])
                    elif k in (2, 4, 6, 8) and bnext < nblk:
                        proj_piece(bnext, k // 2 + 1)
                        tensor.ldweights(sw_sb[:])
                    if t > 0:
                        tensor.wait_ge(s_act, t)
                        mm = tensor.matmul(pslice(t), sw_sb[:],
                                           st[(t - 1) % NSTATE][:],
                                           start=False,
                                           stop=(k == BLK_T - 1),
                                           skip_group_check=True)
                        mm.ins.ldweights = False
                        mm.then_inc(s_pe, 1)

            @block.scalar
            def _(scalar):
                for t in range(T_):
                    if t == 0:
                        scalar.wait_ge(s_proj, 6)
                    else:
                        scalar.wait_ge(s_pe, t)
                    dst = st_f if t == T_ - 1 else st[t % NSTATE]
                    scalar.activation(dst[:], pslice(t), tanh,
                                      bias=b_sb[:, 0:1]).then_inc(s_act, 1)

    nc.move_matmul_waits_to_ldweights = lambda: None
    nc.compile()
    return nc


def _split_bf16(a):
    hi = a.astype(ml_dtypes.bfloat16)
    lo = (a.astype(np.float32) - hi.astype(np.float32)).astype(ml_dtypes.bfloat16)
    return hi, lo


def shard_inputs(x, w, state_weight, b):
    x = np.asarray(x)
    w = np.asarray(w, dtype=np.float32)
    w_hi, w_lo = _split_bf16(w)
    wpack = np.ascontiguousarray(
        np.concatenate([w_hi, w_lo], axis=1))            # [D, 2H]
    sw = np.ascontiguousarray(
        np.asarray(state_weight).astype(ml_dtypes.bfloat16))
    bb = np.ascontiguousarray(np.asarray(b), dtype=np.float32).reshape(H, 1)
    in_maps = []
    for i in range(NCORES):
        xs = np.asarray(x[i * BS:(i + 1) * BS], dtype=np.float32)
        xs = np.ascontiguousarray(xs.transpose(2, 1, 0))  # [D, T, Bs]
        x_hi, x_lo = _split_bf16(xs)
        xpack = np.ascontiguousarray(
            np.stack([x_hi.reshape(D, -1), x_lo.reshape(D, -1)], axis=1))
        in_maps.append({"x": xpack, "w": wpack, "sw": sw, "b": bb})
    return in_maps


_NC = None


def kernel(x, w, state_weight, b, **run_kwargs):
    global _NC
    from concourse.bass_utils import run_bass_kernel_spmd
    if _NC is None:
        _NC = build()
    in_maps = shard_inputs(x, w, state_weight, b)
    res = run_bass_kernel_spmd(_NC, in_maps, core_ids=list(range(NCORES)),
                               **run_kwargs)
    out = np.concatenate([r["out"].T for r in res.results], axis=0)
    if run_kwargs:
        return out, res
    return out
